# revision 1
# baseline (speedup 1.0000x reference)
"""PIoU (pixel-wise IoU) pairwise matrix kernel for Trainium2, 8 NeuronCores.

Math: for each pair (predicted box n, target box m) the reference samples a
16x16 grid of the joint AABB and evaluates a soft membership
F = sigmoid(k(w/2-|A|)) * sigmoid(k(h/2-|B|)) per box, where (A, B) are the
pixel offsets rotated into the box frame.  Both A and B are *affine* in the
grid coordinates (ug, uh), so the sigmoid arguments k(s/2 -+ A) for all
256 pixels x 4 fields x {P,Q} are produced by ONE K=24 matmul per 128 pairs
against a constant basis.  sigmoid(min(P,Q)) == min(sigmoid(P), sigmoid(Q))
lets ACT read the matmul PSUM directly with no bias work.

Sharding: N (predicted) axis split 8 ways; each core computes a [512m, 64n]
slab (output transposed on host).
"""

import numpy as np

N = 512
M = 512
G = 16
NPIX = G * G
K_SLOPE = np.float32(10.0)
EPS = np.float32(1e-6)
NC = 8
NLOC = N // NC  # 64 predicted boxes per core
NCHUNK = 4  # m-chunks of 128

_cache = {}


def _derived(b):
    # b: [K,5] float32 -> per-box derived quantities (all float32)
    cx, cy, w, h, t = (b[:, i].astype(np.float32) for i in range(5))
    c, s = np.cos(t).astype(np.float32), np.sin(t).astype(np.float32)
    hw = np.float32(0.5) * (w * np.abs(c) + h * np.abs(s))
    hh = np.float32(0.5) * (w * np.abs(s) + h * np.abs(c))
    return dict(
        cx=cx, cy=cy, ct=c, st=s,
        khw=(K_SLOPE * np.float32(0.5)) * w, khh=(K_SLOPE * np.float32(0.5)) * h,
        x0=cx - hw, x1=cx + hw, y0=cy - hh, y1=cy + hh,
    )


def _host_constants(loc_p, loc_t):
    """Build per-core input arrays (all O(N+M) host work)."""
    u = ((np.arange(G, dtype=np.float32) + np.float32(0.5)) / np.float32(G))
    Ug = np.tile(u, G)      # pixel p = h*G+g -> u[g]
    Uh = np.repeat(u, G)    # -> u[h]

    # basis [24, 2*NPIX*4]: P-block cols 0..1023 (fields A1,B1,A2,B2 x 256),
    # Q-block cols 1024..2047.  Field f uses rows 3f..3f+2 (P) / 12+3f.. (Q).
    basis = np.zeros((24, 8 * NPIX), dtype=np.float32)
    for f in range(4):
        for blk, r0 in ((0, 0), (1, 12)):
            c0 = blk * 4 * NPIX + f * NPIX
            basis[r0 + 3 * f + 0, c0:c0 + NPIX] = 1.0
            basis[r0 + 3 * f + 1, c0:c0 + NPIX] = Ug
            basis[r0 + 3 * f + 2, c0:c0 + NPIX] = Uh

    T = _derived(loc_t)
    # TQ [128, 4 chunks, 10]: per-target quantities, m = j*128 + partition
    tq_order = ("x0", "x1", "y0", "y1", "cx", "cy", "ct", "st", "khw", "khh")
    TQ = np.empty((128, NCHUNK, len(tq_order)), dtype=np.float32)
    for qi, q in enumerate(tq_order):
        TQ[:, :, qi] = T[q].reshape(NCHUNK, 128).T

    P = _derived(loc_p)
    pb_order = ("x0", "x1", "y0", "y1", "cx", "cy", "ct", "st", "khw", "khh")
    PBs = []
    for c in range(NC):
        sl = slice(c * NLOC, (c + 1) * NLOC)
        pb = np.stack([P[q][sl] for q in pb_order], axis=0)  # [10, 64]
        PBs.append(np.broadcast_to(pb.reshape(1, 10 * NLOC), (128, 10 * NLOC)).copy())
    return basis, TQ.reshape(128, NCHUNK * len(tq_order)), PBs


def _build_nc():
    from contextlib import ExitStack

    import concourse.bacc as bacc
    import concourse.tile as tile
    from concourse import mybir
    from concourse.masks import make_identity

    dt = mybir.dt
    op = mybir.AluOpType
    AF = mybir.ActivationFunctionType
    K = float(K_SLOPE)

    # Bacc (not raw Bass): its finalize() runs generate_event_semaphores,
    # which legalizes Tile's multi-wait sync_info down to <=1 wait per
    # hardware instruction.
    nc = bacc.Bacc(None, target_bir_lowering=False)
    PB_d = nc.declare_dram_parameter("PB", [128, 10 * NLOC], dt.float32, isOutput=False)
    TQ_d = nc.declare_dram_parameter("TQ", [128, NCHUNK * 10], dt.float32, isOutput=False)
    BAS_d = nc.declare_dram_parameter("BASIS", [24, 8 * NPIX], dt.float32, isOutput=False)
    OUT_d = nc.declare_dram_parameter("OUT", [M, NLOC], dt.float32, isOutput=True)

    with tile.TileContext(nc) as tc, ExitStack() as ctx:
        consts = ctx.enter_context(tc.tile_pool(name="consts", bufs=1))
        coeffp = ctx.enter_context(tc.tile_pool(name="coeffp", bufs=2))
        scratch = ctx.enter_context(tc.tile_pool(name="scratch", bufs=2))
        work = ctx.enter_context(tc.tile_pool(name="work", bufs=2))
        accp = ctx.enter_context(tc.tile_pool(name="accp", bufs=2))
        psum = ctx.enter_context(tc.tile_pool(name="psum", bufs=2, space="PSUM"))

        ident = consts.tile([128, 128], dt.float32)
        make_identity(nc, ident[:])
        PB = consts.tile([128, 10, NLOC], dt.float32)
        nc.sync.dma_start(out=PB[:].rearrange("p a b -> p (a b)"), in_=PB_d[:])
        TQ = consts.tile([128, NCHUNK, 10], dt.float32)
        nc.sync.dma_start(out=TQ[:].rearrange("p a b -> p (a b)"), in_=TQ_d[:])
        BAS = consts.tile([24, 8 * NPIX], dt.float32)
        nc.sync.dma_start(out=BAS[:], in_=BAS_d[:])

        def pb(q):
            i = ("x0", "x1", "y0", "y1", "cx", "cy", "ct", "st", "khw", "khh").index(q)
            return PB[:, i, :]

        def tq(j, q):
            i = ("x0", "x1", "y0", "y1", "cx", "cy", "ct", "st", "khw", "khh").index(q)
            return TQ[:, j, i:i + 1]

        for j in range(NCHUNK):
            # ---- coefficient slab C [128 m, 24 rows, 64 n] on GPSIMD ----
            C = coeffp.tile([128, 24, NLOC], dt.float32, tag="C")
            S = scratch.tile([128, 16, NLOC], dt.float32, tag="S")
            g = nc.vector

            def s(i):
                return S[:, i, :]

            if j == 0:
                # DVE instructions carry a single HW sync-wait slot, so the
                # first op after the two input DMAs may not wait on both DMA
                # sems at once.  Chain two single-wait ops; the WAW overlap
                # with s(0) orders the real first op after them with no waits.
                g.tensor_copy(s(0)[:, 1:2], PB[:, 0, 0:1])
                g.tensor_copy(s(0)[:, 0:1], TQ[:, 0, 0:1])

            g.tensor_scalar(s(0), pb("x0"), tq(j, "x0"), None, op.min)   # xmin
            g.tensor_scalar(s(1), pb("x1"), tq(j, "x1"), None, op.max)   # xmax
            g.tensor_scalar(s(2), pb("y0"), tq(j, "y0"), None, op.min)   # ymin
            g.tensor_scalar(s(3), pb("y1"), tq(j, "y1"), None, op.max)   # ymax
            g.tensor_tensor(s(4), s(1), s(0), op.subtract)               # sx
            g.tensor_tensor(s(5), s(3), s(2), op.subtract)               # sy
            g.tensor_tensor(s(6), s(0), pb("cx"), op.subtract)           # dxp
            g.tensor_tensor(s(7), s(2), pb("cy"), op.subtract)           # dyp
            # a0p = dxp*ctp + dyp*stp ; b0p = dyp*ctp - dxp*stp
            g.tensor_tensor(s(8), s(6), pb("ct"), op.mult)
            g.tensor_tensor(s(9), s(7), pb("st"), op.mult)
            g.tensor_tensor(s(9), s(8), s(9), op.add)                    # a0p
            g.tensor_tensor(s(8), s(7), pb("ct"), op.mult)
            g.tensor_tensor(s(10), s(6), pb("st"), op.mult)
            g.tensor_tensor(s(10), s(8), s(10), op.subtract)             # b0p

            def c(r):
                return C[:, r, :]

            # field A1 (const rows): P = khw_p - K*a0p ; Q = khw_p + K*a0p
            g.scalar_tensor_tensor(c(0), s(9), -K, pb("khw"), op.mult, op.add)
            g.scalar_tensor_tensor(c(12), s(9), K, pb("khw"), op.mult, op.add)
            # a1p = sx*ctp -> rows 1/13 ; a2p = sy*stp -> rows 2/14
            g.tensor_tensor(s(8), s(4), pb("ct"), op.mult)
            g.tensor_scalar(c(1), s(8), -K, None, op.mult)
            g.tensor_scalar(c(13), s(8), K, None, op.mult)
            g.tensor_tensor(s(8), s(5), pb("st"), op.mult)
            g.tensor_scalar(c(2), s(8), -K, None, op.mult)
            g.tensor_scalar(c(14), s(8), K, None, op.mult)
            # field B1 (rows 6-8/18-20; field order is A1,A2,B1,B2)
            g.scalar_tensor_tensor(c(6), s(10), -K, pb("khh"), op.mult, op.add)
            g.scalar_tensor_tensor(c(18), s(10), K, pb("khh"), op.mult, op.add)
            # b1p = -sx*stp: s8 = sx*stp -> P row = +K*s8, Q row = -K*s8
            g.tensor_tensor(s(8), s(4), pb("st"), op.mult)
            g.tensor_scalar(c(7), s(8), K, None, op.mult)
            g.tensor_scalar(c(19), s(8), -K, None, op.mult)
            # b2p = sy*ctp
            g.tensor_tensor(s(8), s(5), pb("ct"), op.mult)
            g.tensor_scalar(c(8), s(8), -K, None, op.mult)
            g.tensor_scalar(c(20), s(8), K, None, op.mult)
            # target box: dxt/dyt
            g.tensor_scalar(s(12), s(0), tq(j, "cx"), None, op.subtract)
            g.tensor_scalar(s(13), s(2), tq(j, "cy"), None, op.subtract)
            # a0t = dxt*ctt + dyt*stt
            g.tensor_scalar(s(8), s(12), tq(j, "ct"), None, op.mult)
            g.tensor_scalar(s(14), s(13), tq(j, "st"), None, op.mult)
            g.tensor_tensor(s(14), s(8), s(14), op.add)
            # b0t = dyt*ctt - dxt*stt
            g.tensor_scalar(s(8), s(13), tq(j, "ct"), None, op.mult)
            g.tensor_scalar(s(15), s(12), tq(j, "st"), None, op.mult)
            g.tensor_tensor(s(15), s(8), s(15), op.subtract)
            # field A2 const rows (rows 3-5/15-17)
            g.tensor_scalar(c(3), s(14), -K, tq(j, "khw"), op.mult, op.add)
            g.tensor_scalar(c(15), s(14), K, tq(j, "khw"), op.mult, op.add)
            # a1t = sx*ctt ; a2t = sy*stt
            g.tensor_scalar(s(8), s(4), tq(j, "ct"), None, op.mult)
            g.tensor_scalar(c(4), s(8), -K, None, op.mult)
            g.tensor_scalar(c(16), s(8), K, None, op.mult)
            g.tensor_scalar(s(8), s(5), tq(j, "st"), None, op.mult)
            g.tensor_scalar(c(5), s(8), -K, None, op.mult)
            g.tensor_scalar(c(17), s(8), K, None, op.mult)
            # field B2 const rows
            g.tensor_scalar(c(9), s(15), -K, tq(j, "khh"), op.mult, op.add)
            g.tensor_scalar(c(21), s(15), K, tq(j, "khh"), op.mult, op.add)
            # b1t = -sx*stt ; b2t = sy*ctt
            g.tensor_scalar(s(8), s(4), tq(j, "st"), None, op.mult)
            g.tensor_scalar(c(10), s(8), K, None, op.mult)
            g.tensor_scalar(c(22), s(8), -K, None, op.mult)
            g.tensor_scalar(s(8), s(5), tq(j, "ct"), None, op.mult)
            g.tensor_scalar(c(11), s(8), -K, None, op.mult)
            g.tensor_scalar(c(23), s(8), K, None, op.mult)

            Ssum = accp.tile([128, NLOC], dt.float32, tag="Ssum")
            Isum = accp.tile([128, NLOC], dt.float32, tag="Isum")

            # ---- main loop over the 64 predicted boxes of this core ----
            for n in range(NLOC):
                coeffT = psum.tile([24, 128], dt.float32, tag="coeffT")
                nc.tensor.transpose(coeffT[:], C[:, :, n], ident[:])
                lhsT = work.tile([24, 128], dt.float32, tag="lhsT")
                nc.vector.tensor_copy(lhsT[:], coeffT[:])

                fieldP = psum.tile([128, 4 * NPIX], dt.float32, tag="fields", bufs=3)
                fieldQ = psum.tile([128, 4 * NPIX], dt.float32, tag="fields", bufs=3)
                if j == 0 and n == 0:
                    # Warm the PE clock on the BAS DMA sem (single-wait LDW)
                    # before the first real matmul, which must wait on the
                    # DVE-written lhsT.  WAW into fieldP orders it first.
                    nc.tensor.transpose(fieldP[:, 0:24], BAS[0:24, 0:128], ident[0:24, 0:24])
                for q in range(2):
                    nc.tensor.matmul(
                        fieldP[:, q * 512:(q + 1) * 512],
                        lhsT[:], BAS[:, q * 512:(q + 1) * 512],
                        start=True, stop=True)
                for q in range(2):
                    nc.tensor.matmul(
                        fieldQ[:, q * 512:(q + 1) * 512],
                        lhsT[:], BAS[:, 1024 + q * 512:1024 + (q + 1) * 512],
                        start=True, stop=True)
                sigP = work.tile([128, 4 * NPIX], dt.bfloat16, tag="sigP")
                nc.scalar.activation(sigP[:], fieldP[:], AF.Sigmoid)
                sigQ = work.tile([128, 4 * NPIX], dt.bfloat16, tag="sigQ")
                nc.scalar.activation(sigQ[:], fieldQ[:], AF.Sigmoid)

                vmin = work.tile([128, 4, NPIX], dt.bfloat16, tag="vmin")
                nc.vector.tensor_tensor(
                    vmin[:].rearrange("p f q -> p (f q)"),
                    sigP[:], sigQ[:], op.min)

                vflat = vmin[:].rearrange("p f q -> p (f q)")
                Fp = work.tile([128, 2 * NPIX], dt.bfloat16, tag="Fp")
                nc.vector.tensor_mul(Fp[:], vflat[:, 0:2 * NPIX], vflat[:, 2 * NPIX:4 * NPIX])
                nc.vector.tensor_reduce(
                    Ssum[:, n:n + 1], Fp[:], mybir.AxisListType.X, op.add)
                F12 = work.tile([128, NPIX], dt.bfloat16, tag="F12")
                nc.vector.tensor_mul(F12[:], Fp[:, 0:NPIX], Fp[:, NPIX:2 * NPIX])
                nc.vector.tensor_reduce(
                    Isum[:, n:n + 1], F12[:], mybir.AxisListType.X, op.add)

            # ---- epilogue: piou = inter / (stot - inter + eps) ----
            union = scratch.tile([128, NLOC], dt.float32, tag="union")
            nc.vector.scalar_tensor_tensor(
                union[:], Isum[:], -1.0, Ssum[:], op.mult, op.add)
            nc.vector.tensor_scalar(union[:], union[:], float(EPS), None, op.add)
            rec = scratch.tile([128, NLOC], dt.float32, tag="rec")
            nc.vector.reciprocal(rec[:], union[:])
            piou = accp.tile([128, NLOC], dt.float32, tag="piou")
            nc.vector.tensor_tensor(piou[:], Isum[:], rec[:], op.mult)
            nc.sync.dma_start(out=OUT_d[j * 128:(j + 1) * 128, :], in_=piou[:])

    nc.finalize()
    return nc


def _get_compiled():
    if "nc" not in _cache:
        _cache["nc"] = _build_nc()
    return _cache["nc"]


def kernel(loc_p, loc_t, grid):
    from concourse.bass_utils import run_bass_kernel_spmd

    assert int(grid) == G
    loc_p = np.asarray(loc_p, dtype=np.float32)
    loc_t = np.asarray(loc_t, dtype=np.float32)
    basis, TQ, PBs = _host_constants(loc_p, loc_t)

    nc = _get_compiled()
    in_maps = [{"PB": PBs[c], "TQ": TQ, "BASIS": basis} for c in range(NC)]
    res = run_bass_kernel_spmd(nc, in_maps, core_ids=list(range(NC)))
    out = np.empty((N, M), dtype=np.float32)
    for c in range(NC):
        out[c * NLOC:(c + 1) * NLOC, :] = res.results[c]["OUT"].T
    return out



# revision 3
# speedup vs baseline: 7.9625x; 7.9625x over previous
"""PIoU (pixel-wise IoU) pairwise matrix kernel for Trainium2, 8 NeuronCores.

Math: for each pair (predicted box n, target box m) the reference samples a
16x16 grid of the joint AABB and evaluates a soft membership
F = sigmoid(k(w/2-|A|)) * sigmoid(k(h/2-|B|)) per box, where (A, B) are the
pixel offsets rotated into the box frame.  Both A and B are *affine* in the
grid coordinates (ug, uh), so the sigmoid arguments k(s/2 -+ A) for all
256 pixels x 4 fields x {P,Q} are produced by ONE K=24 matmul per 128 pairs
against a constant basis.  sigmoid(min(P,Q)) == min(sigmoid(P), sigmoid(Q))
lets ACT read the matmul PSUM directly with no bias work.

Sharding: N (predicted) axis split 8 ways; each core computes a [512m, 64n]
slab (output transposed on host).

Dispatch: the per-call cost is dominated by the axon tunnel round trip
(~80ms) plus payload bytes, so the runner (a) jits the PJRT shard_map
function ONCE and reuses it every call, (b) keeps the constant basis and
the structural zero-output buffers resident on device, and (c) ships only
one packed [144,40] f32 tensor per core per call (~184KB total) with the
per-box broadcast done on device via a K=1 matmul against an all-ones
basis row.  Output returns as fp16 (512KB) and is upcast on host.
"""

import numpy as np

N = 512
M = 512
G = 16
NPIX = G * G
K_SLOPE = np.float32(10.0)
EPS = np.float32(1e-6)
NC = 8
NLOC = N // NC  # 64 predicted boxes per core
NCHUNK = 4  # m-chunks of 128
XROWS = 144  # packed input: rows 0..127 = TQ [128,40], rows 128..143 = PB [16,40]

_cache = {}

_Q_ORDER = ("x0", "x1", "y0", "y1", "cx", "cy", "ct", "st", "khw", "khh")


def _derived(b):
    # b: [K,5] float32 -> per-box derived quantities (all float32)
    cx, cy, w, h, t = (b[:, i].astype(np.float32) for i in range(5))
    c, s = np.cos(t).astype(np.float32), np.sin(t).astype(np.float32)
    hw = np.float32(0.5) * (w * np.abs(c) + h * np.abs(s))
    hh = np.float32(0.5) * (w * np.abs(s) + h * np.abs(c))
    return dict(
        cx=cx, cy=cy, ct=c, st=s,
        khw=(K_SLOPE * np.float32(0.5)) * w, khh=(K_SLOPE * np.float32(0.5)) * h,
        x0=cx - hw, x1=cx + hw, y0=cy - hh, y1=cy + hh,
    )


def _basis():
    """Constant [24, 2*NPIX*4] sampling basis (shipped to device once)."""
    if "basis" in _cache:
        return _cache["basis"]
    u = ((np.arange(G, dtype=np.float32) + np.float32(0.5)) / np.float32(G))
    Ug = np.tile(u, G)      # pixel p = h*G+g -> u[g]
    Uh = np.repeat(u, G)    # -> u[h]

    # basis [24, 2*NPIX*4]: P-block cols 0..1023 (fields A1,B1,A2,B2 x 256),
    # Q-block cols 1024..2047.  Field f uses rows 3f..3f+2 (P) / 12+3f.. (Q).
    # Row 0 is all-ones over cols 0..255; its first 128 cols double as the
    # lhsT for the on-device partition broadcast of the per-box rows.
    basis = np.zeros((24, 8 * NPIX), dtype=np.float32)
    for f in range(4):
        for blk, r0 in ((0, 0), (1, 12)):
            c0 = blk * 4 * NPIX + f * NPIX
            basis[r0 + 3 * f + 0, c0:c0 + NPIX] = 1.0
            basis[r0 + 3 * f + 1, c0:c0 + NPIX] = Ug
            basis[r0 + 3 * f + 2, c0:c0 + NPIX] = Uh
    _cache["basis"] = basis
    return basis


def _pack_inputs(loc_p, loc_t):
    """Per-call packed input [NC*XROWS, 40]: TQ (replicated) + per-core PB."""
    T = _derived(loc_t)
    # TQ [128, 4 chunks, 10]: per-target quantities, m = j*128 + partition
    TQ = np.empty((128, NCHUNK, len(_Q_ORDER)), dtype=np.float32)
    for qi, q in enumerate(_Q_ORDER):
        TQ[:, :, qi] = T[q].reshape(NCHUNK, 128).T
    TQflat = TQ.reshape(128, NCHUNK * len(_Q_ORDER))

    P = _derived(loc_p)
    X = np.empty((NC, XROWS, 40), dtype=np.float32)
    for c in range(NC):
        sl = slice(c * NLOC, (c + 1) * NLOC)
        pb = np.stack([P[q][sl] for q in _Q_ORDER], axis=0)  # [10, 64]
        X[c, 0:128, :] = TQflat
        X[c, 128:XROWS, :] = pb.reshape(16, 40)
    return X.reshape(NC * XROWS, 40)


def _build_nc():
    from contextlib import ExitStack

    import concourse.bacc as bacc
    import concourse.tile as tile
    from concourse import mybir
    from concourse.masks import make_identity

    dt = mybir.dt
    op = mybir.AluOpType
    AF = mybir.ActivationFunctionType
    K = float(K_SLOPE)

    # Bacc (not raw Bass): its finalize() runs generate_event_semaphores,
    # which legalizes Tile's multi-wait sync_info down to <=1 wait per
    # hardware instruction.
    nc = bacc.Bacc(None, target_bir_lowering=False)
    X_d = nc.declare_dram_parameter("X", [XROWS, 40], dt.float32, isOutput=False)
    BAS_d = nc.declare_dram_parameter("BASIS", [24, 8 * NPIX], dt.float32, isOutput=False)
    OUT_d = nc.declare_dram_parameter("OUT", [M, NLOC], dt.float16, isOutput=True)

    with tile.TileContext(nc) as tc, ExitStack() as ctx:
        consts = ctx.enter_context(tc.tile_pool(name="consts", bufs=1))
        coeffp = ctx.enter_context(tc.tile_pool(name="coeffp", bufs=2))
        scratch = ctx.enter_context(tc.tile_pool(name="scratch", bufs=2))
        work = ctx.enter_context(tc.tile_pool(name="work", bufs=2))
        accp = ctx.enter_context(tc.tile_pool(name="accp", bufs=2))
        psum = ctx.enter_context(tc.tile_pool(name="psum", bufs=2, space="PSUM"))

        ident = consts.tile([128, 128], dt.float32)
        make_identity(nc, ident[:])
        TQ = consts.tile([128, NCHUNK, 10], dt.float32)
        nc.sync.dma_start(out=TQ[:].rearrange("p a b -> p (a b)"), in_=X_d[0:128, :])
        pbrow = consts.tile([1, 10 * NLOC], dt.float32)
        nc.sync.dma_start(
            out=pbrow[:],
            in_=X_d[128:XROWS, :].rearrange("a b -> (a b)").unsqueeze(0))
        BAS = consts.tile([24, 8 * NPIX], dt.float32)
        nc.sync.dma_start(out=BAS[:], in_=BAS_d[:])

        # Broadcast the 640 per-box values to all 128 partitions with a K=1
        # matmul: BAS row 0 is all-ones on cols 0..255, so BAS[0:1, 0:128]
        # is a ready-made ones lhsT.  PSUM reuses one "fields" ring slot.
        PB = consts.tile([128, 10, NLOC], dt.float32)
        PBflat = PB[:].rearrange("p a b -> p (a b)")
        bc = psum.tile([128, 4 * NPIX], dt.float32, tag="fields", bufs=3)
        nc.tensor.matmul(bc[:, 0:512], BAS[0:1, 0:128], pbrow[:, 0:512],
                         start=True, stop=True)
        nc.tensor.matmul(bc[:, 512:640], BAS[0:1, 0:128], pbrow[:, 512:640],
                         start=True, stop=True)
        nc.vector.tensor_copy(PBflat[:, 0:640], bc[:, 0:640])

        def pb(q):
            return PB[:, _Q_ORDER.index(q), :]

        def tq(j, q):
            i = _Q_ORDER.index(q)
            return TQ[:, j, i:i + 1]

        for j in range(NCHUNK):
            # ---- coefficient slab C [128 m, 24 rows, 64 n] on DVE ----
            C = coeffp.tile([128, 24, NLOC], dt.float32, tag="C")
            S = scratch.tile([128, 16, NLOC], dt.float32, tag="S")
            g = nc.vector

            def s(i):
                return S[:, i, :]

            g.tensor_scalar(s(0), pb("x0"), tq(j, "x0"), None, op.min)   # xmin
            g.tensor_scalar(s(1), pb("x1"), tq(j, "x1"), None, op.max)   # xmax
            g.tensor_scalar(s(2), pb("y0"), tq(j, "y0"), None, op.min)   # ymin
            g.tensor_scalar(s(3), pb("y1"), tq(j, "y1"), None, op.max)   # ymax
            g.tensor_tensor(s(4), s(1), s(0), op.subtract)               # sx
            g.tensor_tensor(s(5), s(3), s(2), op.subtract)               # sy
            g.tensor_tensor(s(6), s(0), pb("cx"), op.subtract)           # dxp
            g.tensor_tensor(s(7), s(2), pb("cy"), op.subtract)           # dyp
            # a0p = dxp*ctp + dyp*stp ; b0p = dyp*ctp - dxp*stp
            g.tensor_tensor(s(8), s(6), pb("ct"), op.mult)
            g.tensor_tensor(s(9), s(7), pb("st"), op.mult)
            g.tensor_tensor(s(9), s(8), s(9), op.add)                    # a0p
            g.tensor_tensor(s(8), s(7), pb("ct"), op.mult)
            g.tensor_tensor(s(10), s(6), pb("st"), op.mult)
            g.tensor_tensor(s(10), s(8), s(10), op.subtract)             # b0p

            def c(r):
                return C[:, r, :]

            # field A1 (const rows): P = khw_p - K*a0p ; Q = khw_p + K*a0p
            g.scalar_tensor_tensor(c(0), s(9), -K, pb("khw"), op.mult, op.add)
            g.scalar_tensor_tensor(c(12), s(9), K, pb("khw"), op.mult, op.add)
            # a1p = sx*ctp -> rows 1/13 ; a2p = sy*stp -> rows 2/14
            g.tensor_tensor(s(8), s(4), pb("ct"), op.mult)
            g.tensor_scalar(c(1), s(8), -K, None, op.mult)
            g.tensor_scalar(c(13), s(8), K, None, op.mult)
            g.tensor_tensor(s(8), s(5), pb("st"), op.mult)
            g.tensor_scalar(c(2), s(8), -K, None, op.mult)
            g.tensor_scalar(c(14), s(8), K, None, op.mult)
            # field B1 (rows 6-8/18-20; field order is A1,A2,B1,B2)
            g.scalar_tensor_tensor(c(6), s(10), -K, pb("khh"), op.mult, op.add)
            g.scalar_tensor_tensor(c(18), s(10), K, pb("khh"), op.mult, op.add)
            # b1p = -sx*stp: s8 = sx*stp -> P row = +K*s8, Q row = -K*s8
            g.tensor_tensor(s(8), s(4), pb("st"), op.mult)
            g.tensor_scalar(c(7), s(8), K, None, op.mult)
            g.tensor_scalar(c(19), s(8), -K, None, op.mult)
            # b2p = sy*ctp
            g.tensor_tensor(s(8), s(5), pb("ct"), op.mult)
            g.tensor_scalar(c(8), s(8), -K, None, op.mult)
            g.tensor_scalar(c(20), s(8), K, None, op.mult)
            # target box: dxt/dyt
            g.tensor_scalar(s(12), s(0), tq(j, "cx"), None, op.subtract)
            g.tensor_scalar(s(13), s(2), tq(j, "cy"), None, op.subtract)
            # a0t = dxt*ctt + dyt*stt
            g.tensor_scalar(s(8), s(12), tq(j, "ct"), None, op.mult)
            g.tensor_scalar(s(14), s(13), tq(j, "st"), None, op.mult)
            g.tensor_tensor(s(14), s(8), s(14), op.add)
            # b0t = dyt*ctt - dxt*stt
            g.tensor_scalar(s(8), s(13), tq(j, "ct"), None, op.mult)
            g.tensor_scalar(s(15), s(12), tq(j, "st"), None, op.mult)
            g.tensor_tensor(s(15), s(8), s(15), op.subtract)
            # field A2 const rows (rows 3-5/15-17)
            g.tensor_scalar(c(3), s(14), -K, tq(j, "khw"), op.mult, op.add)
            g.tensor_scalar(c(15), s(14), K, tq(j, "khw"), op.mult, op.add)
            # a1t = sx*ctt ; a2t = sy*stt
            g.tensor_scalar(s(8), s(4), tq(j, "ct"), None, op.mult)
            g.tensor_scalar(c(4), s(8), -K, None, op.mult)
            g.tensor_scalar(c(16), s(8), K, None, op.mult)
            g.tensor_scalar(s(8), s(5), tq(j, "st"), None, op.mult)
            g.tensor_scalar(c(5), s(8), -K, None, op.mult)
            g.tensor_scalar(c(17), s(8), K, None, op.mult)
            # field B2 const rows
            g.tensor_scalar(c(9), s(15), -K, tq(j, "khh"), op.mult, op.add)
            g.tensor_scalar(c(21), s(15), K, tq(j, "khh"), op.mult, op.add)
            # b1t = -sx*stt ; b2t = sy*ctt
            g.tensor_scalar(s(8), s(4), tq(j, "st"), None, op.mult)
            g.tensor_scalar(c(10), s(8), K, None, op.mult)
            g.tensor_scalar(c(22), s(8), -K, None, op.mult)
            g.tensor_scalar(s(8), s(5), tq(j, "ct"), None, op.mult)
            g.tensor_scalar(c(11), s(8), -K, None, op.mult)
            g.tensor_scalar(c(23), s(8), K, None, op.mult)

            Ssum = accp.tile([128, NLOC], dt.float32, tag="Ssum")
            Isum = accp.tile([128, NLOC], dt.float32, tag="Isum")

            # ---- main loop over the 64 predicted boxes of this core ----
            for n in range(NLOC):
                coeffT = psum.tile([24, 128], dt.float32, tag="coeffT")
                nc.tensor.transpose(coeffT[:], C[:, :, n], ident[:])
                lhsT = work.tile([24, 128], dt.float32, tag="lhsT")
                nc.vector.tensor_copy(lhsT[:], coeffT[:])

                fieldP = psum.tile([128, 4 * NPIX], dt.float32, tag="fields", bufs=3)
                fieldQ = psum.tile([128, 4 * NPIX], dt.float32, tag="fields", bufs=3)
                for q in range(2):
                    nc.tensor.matmul(
                        fieldP[:, q * 512:(q + 1) * 512],
                        lhsT[:], BAS[:, q * 512:(q + 1) * 512],
                        start=True, stop=True)
                for q in range(2):
                    nc.tensor.matmul(
                        fieldQ[:, q * 512:(q + 1) * 512],
                        lhsT[:], BAS[:, 1024 + q * 512:1024 + (q + 1) * 512],
                        start=True, stop=True)
                sigP = work.tile([128, 4 * NPIX], dt.bfloat16, tag="sigP")
                nc.scalar.activation(sigP[:], fieldP[:], AF.Sigmoid)
                sigQ = work.tile([128, 4 * NPIX], dt.bfloat16, tag="sigQ")
                nc.scalar.activation(sigQ[:], fieldQ[:], AF.Sigmoid)

                vmin = work.tile([128, 4, NPIX], dt.bfloat16, tag="vmin")
                nc.vector.tensor_tensor(
                    vmin[:].rearrange("p f q -> p (f q)"),
                    sigP[:], sigQ[:], op.min)

                vflat = vmin[:].rearrange("p f q -> p (f q)")
                Fp = work.tile([128, 2 * NPIX], dt.bfloat16, tag="Fp")
                nc.vector.tensor_mul(Fp[:], vflat[:, 0:2 * NPIX], vflat[:, 2 * NPIX:4 * NPIX])
                nc.vector.tensor_reduce(
                    Ssum[:, n:n + 1], Fp[:], mybir.AxisListType.X, op.add)
                F12 = work.tile([128, NPIX], dt.bfloat16, tag="F12")
                nc.vector.tensor_mul(F12[:], Fp[:, 0:NPIX], Fp[:, NPIX:2 * NPIX])
                nc.vector.tensor_reduce(
                    Isum[:, n:n + 1], F12[:], mybir.AxisListType.X, op.add)

            # ---- epilogue: piou = inter / (stot - inter + eps) ----
            union = scratch.tile([128, NLOC], dt.float32, tag="union")
            nc.vector.scalar_tensor_tensor(
                union[:], Isum[:], -1.0, Ssum[:], op.mult, op.add)
            nc.vector.tensor_scalar(union[:], union[:], float(EPS), None, op.add)
            rec = scratch.tile([128, NLOC], dt.float32, tag="rec")
            nc.vector.reciprocal(rec[:], union[:])
            piou = accp.tile([128, NLOC], dt.float16, tag="piou")
            nc.vector.tensor_tensor(piou[:], Isum[:], rec[:], op.mult)
            nc.sync.dma_start(out=OUT_d[j * 128:(j + 1) * 128, :], in_=piou[:])

    nc.finalize()
    return nc


def _get_runner():
    """Build (once) a reusable jitted PJRT dispatch for the Bass module.

    Re-jitting per call (what run_bass_kernel_spmd does under axon) costs
    ~400ms of retrace + XLA recompile every call; here the jitted callable,
    the device-resident BASIS constant, and the structural zero-output
    buffers all persist across calls.
    """
    if "runner" in _cache:
        return _cache["runner"]

    import jax
    from jax.sharding import Mesh, NamedSharding, PartitionSpec
    from jax.experimental.shard_map import shard_map  # check_rep kwarg
    from concourse import mybir
    from concourse.bass2jax import (
        _bass_exec_p,
        install_neuronx_cc_hook,
        partition_id_tensor,
    )

    nc = _build_nc()
    install_neuronx_cc_hook()
    partition_name = nc.partition_id_tensor.name if nc.partition_id_tensor else None

    in_names, out_names, out_avals = [], [], []
    for alloc in nc.m.functions[0].allocations:
        if not isinstance(alloc, mybir.MemoryLocationSet):
            continue
        name = alloc.memorylocations[0].name
        if alloc.kind == "ExternalInput":
            if name != partition_name:
                in_names.append(name)
        elif alloc.kind == "ExternalOutput":
            out_names.append(name)
            out_avals.append(
                jax.core.ShapedArray(tuple(alloc.tensor_shape),
                                     mybir.dt.np(alloc.dtype)))
    assert in_names == ["X", "BASIS"] and out_names == ["OUT"], (in_names, out_names)

    all_in_names = list(in_names) + list(out_names)
    if partition_name is not None:
        all_in_names.append(partition_name)

    def _body(*args):
        operands = list(args)
        if partition_name is not None:
            operands.append(partition_id_tensor())
        return tuple(_bass_exec_p.bind(
            *operands,
            out_avals=tuple(out_avals),
            in_names=tuple(all_in_names),
            out_names=tuple(out_names),
            lowering_input_output_aliases=(),
            sim_require_finite=True,
            sim_require_nnan=True,
            nc=nc,
        ))

    mesh = Mesh(np.asarray(jax.devices()[:NC]), ("core",))
    spec = PartitionSpec("core")
    n_ops = len(in_names) + len(out_names)
    fn = jax.jit(
        shard_map(_body, mesh=mesh, in_specs=(spec,) * n_ops,
                  out_specs=(spec,) * len(out_names), check_rep=False),
        keep_unused=True,
    )

    sh = NamedSharding(mesh, spec)
    # Device-resident across calls: the constant basis (replicated per core)
    # and the zero buffer backing the OUT operand slot.  The NEFF never
    # reads or writes this operand (the renamed NEFF binds OUT only as
    # output0, and the kernel writes every OUT element), so one buffer can
    # be reused every call without donation.
    bas_dev = jax.device_put(
        np.broadcast_to(_basis(), (NC, 24, 8 * NPIX)).reshape(NC * 24, 8 * NPIX), sh)
    zout_dev = jax.device_put(np.zeros((NC * M, NLOC), np.float16), sh)

    def run(x_packed):
        out, = fn(x_packed, bas_dev, zout_dev)
        return np.asarray(out)  # [NC*M, NLOC] fp16

    # Warm: pay NEFF compile + jit trace + device layout here, not in the
    # first timed call.
    run(np.zeros((NC * XROWS, 40), np.float32))

    _cache["runner"] = run
    return run


def kernel(loc_p, loc_t, grid):
    assert int(grid) == G
    loc_p = np.asarray(loc_p, dtype=np.float32)
    loc_t = np.asarray(loc_t, dtype=np.float32)

    run = _get_runner()
    res = run(_pack_inputs(loc_p, loc_t))  # [NC*M, NLOC] fp16
    # res[c*M + m, n] = piou(box c*NLOC+n, box m)  ->  out[n_global, m]
    out = res.reshape(NC, M, NLOC).transpose(0, 2, 1).reshape(N, M)
    return np.ascontiguousarray(out, dtype=np.float32)


# revision 4
# speedup vs baseline: 9.2399x; 1.1604x over previous
"""PIoU (pixel-wise IoU) pairwise matrix kernel for Trainium2, 8 NeuronCores.

Math: for each pair (predicted box n, target box m) the reference samples a
16x16 grid of the joint AABB and evaluates a soft membership
F = sigmoid(k(w/2-|A|)) * sigmoid(k(h/2-|B|)) per box, where (A, B) are the
pixel offsets rotated into the box frame.  Both A and B are *affine* in the
grid coordinates (ug, uh), so the sigmoid arguments k(s/2 -+ A) for all
256 pixels x 4 fields x {P,Q} are produced by ONE K=24 matmul per 128 pairs
against a constant basis.  sigmoid(min(P,Q)) == min(sigmoid(P), sigmoid(Q))
lets ACT read the matmul PSUM directly with no bias work.

Sharding: N (predicted) axis split 8 ways; each core computes a [512m, 64n]
slab (output transposed on host).

Dispatch: the per-call cost is dominated by the axon tunnel round trip
(~80ms) plus payload bytes, so the runner (a) jits the PJRT shard_map
function ONCE and reuses it every call, (b) keeps the constant basis and
the structural zero-output buffers resident on device, and (c) ships only
one packed [144,40] f32 tensor per core per call (~184KB total) with the
per-box broadcast done on device via a K=1 matmul against an all-ones
basis row.  Output returns as fp16 (512KB) and is upcast on host.
"""

import numpy as np

N = 512
M = 512
G = 16
NPIX = G * G
K_SLOPE = np.float32(10.0)
EPS = np.float32(1e-6)
NC = 8
NLOC = N // NC  # 64 predicted boxes per core
NCHUNK = 4  # m-chunks of 128
XROWS = 144  # packed input: rows 0..127 = TQ [128,40], rows 128..143 = PB [16,40]

_cache = {}

_Q_ORDER = ("x0", "x1", "y0", "y1", "cx", "cy", "ct", "st", "khw", "khh")


def _derived(b):
    # b: [K,5] float32 -> per-box derived quantities (all float32)
    cx, cy, w, h, t = (b[:, i].astype(np.float32) for i in range(5))
    c, s = np.cos(t).astype(np.float32), np.sin(t).astype(np.float32)
    hw = np.float32(0.5) * (w * np.abs(c) + h * np.abs(s))
    hh = np.float32(0.5) * (w * np.abs(s) + h * np.abs(c))
    return dict(
        cx=cx, cy=cy, ct=c, st=s,
        khw=(K_SLOPE * np.float32(0.5)) * w, khh=(K_SLOPE * np.float32(0.5)) * h,
        x0=cx - hw, x1=cx + hw, y0=cy - hh, y1=cy + hh,
    )


def _basis():
    """Constant [24, 2*NPIX*4] sampling basis (shipped to device once)."""
    if "basis" in _cache:
        return _cache["basis"]
    u = ((np.arange(G, dtype=np.float32) + np.float32(0.5)) / np.float32(G))
    Ug = np.tile(u, G)      # pixel p = h*G+g -> u[g]
    Uh = np.repeat(u, G)    # -> u[h]

    # basis [24, 2*NPIX*4]: P-block cols 0..1023 (fields A1,B1,A2,B2 x 256),
    # Q-block cols 1024..2047.  Field f uses rows 3f..3f+2 (P) / 12+3f.. (Q).
    # Row 0 is all-ones over cols 0..255; its first 128 cols double as the
    # lhsT for the on-device partition broadcast of the per-box rows.
    basis = np.zeros((24, 8 * NPIX), dtype=np.float32)
    for f in range(4):
        for blk, r0 in ((0, 0), (1, 12)):
            c0 = blk * 4 * NPIX + f * NPIX
            basis[r0 + 3 * f + 0, c0:c0 + NPIX] = 1.0
            basis[r0 + 3 * f + 1, c0:c0 + NPIX] = Ug
            basis[r0 + 3 * f + 2, c0:c0 + NPIX] = Uh
    _cache["basis"] = basis
    return basis


def _pack_inputs(loc_p, loc_t):
    """Per-call packed input [NC*XROWS, 40]: TQ (replicated) + per-core PB."""
    T = _derived(loc_t)
    # TQ [128, 4 chunks, 10]: per-target quantities, m = j*128 + partition
    TQ = np.empty((128, NCHUNK, len(_Q_ORDER)), dtype=np.float32)
    for qi, q in enumerate(_Q_ORDER):
        TQ[:, :, qi] = T[q].reshape(NCHUNK, 128).T
    TQflat = TQ.reshape(128, NCHUNK * len(_Q_ORDER))

    P = _derived(loc_p)
    X = np.empty((NC, XROWS, 40), dtype=np.float32)
    for c in range(NC):
        sl = slice(c * NLOC, (c + 1) * NLOC)
        pb = np.stack([P[q][sl] for q in _Q_ORDER], axis=0)  # [10, 64]
        X[c, 0:128, :] = TQflat
        X[c, 128:XROWS, :] = pb.reshape(16, 40)
    return X.reshape(NC * XROWS, 40)


def _build_nc():
    from contextlib import ExitStack

    import concourse.bacc as bacc
    import concourse.tile as tile
    from concourse import mybir
    from concourse.masks import make_identity

    dt = mybir.dt
    op = mybir.AluOpType
    AF = mybir.ActivationFunctionType
    K = float(K_SLOPE)

    # Bacc (not raw Bass): its finalize() runs generate_event_semaphores,
    # which legalizes Tile's multi-wait sync_info down to <=1 wait per
    # hardware instruction.
    nc = bacc.Bacc(None, target_bir_lowering=False)
    X_d = nc.declare_dram_parameter("X", [XROWS, 40], dt.float32, isOutput=False)
    BAS_d = nc.declare_dram_parameter("BASIS", [24, 8 * NPIX], dt.float32, isOutput=False)
    OUT_d = nc.declare_dram_parameter("OUT", [M, NLOC], dt.float16, isOutput=True)

    with tile.TileContext(nc) as tc, ExitStack() as ctx:
        consts = ctx.enter_context(tc.tile_pool(name="consts", bufs=1))
        coeffp = ctx.enter_context(tc.tile_pool(name="coeffp", bufs=2))
        scratch = ctx.enter_context(tc.tile_pool(name="scratch", bufs=2))
        work = ctx.enter_context(tc.tile_pool(name="work", bufs=2))
        accp = ctx.enter_context(tc.tile_pool(name="accp", bufs=2))
        psum = ctx.enter_context(tc.tile_pool(name="psum", bufs=2, space="PSUM"))

        ident = consts.tile([128, 128], dt.float32)
        make_identity(nc, ident[:])
        TQ = consts.tile([128, NCHUNK, 10], dt.float32)
        nc.sync.dma_start(out=TQ[:].rearrange("p a b -> p (a b)"), in_=X_d[0:128, :])
        pbrow = consts.tile([1, 10 * NLOC], dt.float32)
        nc.sync.dma_start(
            out=pbrow[:],
            in_=X_d[128:XROWS, :].rearrange("a b -> (a b)").unsqueeze(0))
        BAS = consts.tile([24, 8 * NPIX], dt.float32)
        nc.sync.dma_start(out=BAS[:], in_=BAS_d[:])

        # Broadcast the 640 per-box values to all 128 partitions with a K=1
        # matmul: BAS row 0 is all-ones on cols 0..255, so BAS[0:1, 0:128]
        # is a ready-made ones lhsT.  PSUM reuses one "fields" ring slot.
        PB = consts.tile([128, 10, NLOC], dt.float32)
        PBflat = PB[:].rearrange("p a b -> p (a b)")
        bc = psum.tile([128, 4 * NPIX], dt.float32, tag="fields", bufs=3)
        nc.tensor.matmul(bc[:, 0:512], BAS[0:1, 0:128], pbrow[:, 0:512],
                         start=True, stop=True)
        nc.tensor.matmul(bc[:, 512:640], BAS[0:1, 0:128], pbrow[:, 512:640],
                         start=True, stop=True)
        nc.vector.tensor_copy(PBflat[:, 0:640], bc[:, 0:640])

        def pb(q):
            return PB[:, _Q_ORDER.index(q), :]

        def tq(j, q):
            i = _Q_ORDER.index(q)
            return TQ[:, j, i:i + 1]

        for j in range(NCHUNK):
            # ---- coefficient slab C [128 m, 24 rows, 64 n] on DVE ----
            C = coeffp.tile([128, 24, NLOC], dt.float32, tag="C")
            S = scratch.tile([128, 16, NLOC], dt.float32, tag="S")
            g = nc.vector

            def s(i):
                return S[:, i, :]

            g.tensor_scalar(s(0), pb("x0"), tq(j, "x0"), None, op.min)   # xmin
            g.tensor_scalar(s(1), pb("x1"), tq(j, "x1"), None, op.max)   # xmax
            g.tensor_scalar(s(2), pb("y0"), tq(j, "y0"), None, op.min)   # ymin
            g.tensor_scalar(s(3), pb("y1"), tq(j, "y1"), None, op.max)   # ymax
            g.tensor_tensor(s(4), s(1), s(0), op.subtract)               # sx
            g.tensor_tensor(s(5), s(3), s(2), op.subtract)               # sy
            g.tensor_tensor(s(6), s(0), pb("cx"), op.subtract)           # dxp
            g.tensor_tensor(s(7), s(2), pb("cy"), op.subtract)           # dyp
            # a0p = dxp*ctp + dyp*stp ; b0p = dyp*ctp - dxp*stp
            g.tensor_tensor(s(8), s(6), pb("ct"), op.mult)
            g.tensor_tensor(s(9), s(7), pb("st"), op.mult)
            g.tensor_tensor(s(9), s(8), s(9), op.add)                    # a0p
            g.tensor_tensor(s(8), s(7), pb("ct"), op.mult)
            g.tensor_tensor(s(10), s(6), pb("st"), op.mult)
            g.tensor_tensor(s(10), s(8), s(10), op.subtract)             # b0p

            def c(r):
                return C[:, r, :]

            # field A1 (const rows): P = khw_p - K*a0p ; Q = khw_p + K*a0p
            g.scalar_tensor_tensor(c(0), s(9), -K, pb("khw"), op.mult, op.add)
            g.scalar_tensor_tensor(c(12), s(9), K, pb("khw"), op.mult, op.add)
            # a1p = sx*ctp -> rows 1/13 ; a2p = sy*stp -> rows 2/14
            g.tensor_tensor(s(8), s(4), pb("ct"), op.mult)
            g.tensor_scalar(c(1), s(8), -K, None, op.mult)
            g.tensor_scalar(c(13), s(8), K, None, op.mult)
            g.tensor_tensor(s(8), s(5), pb("st"), op.mult)
            g.tensor_scalar(c(2), s(8), -K, None, op.mult)
            g.tensor_scalar(c(14), s(8), K, None, op.mult)
            # field B1 (rows 6-8/18-20; field order is A1,A2,B1,B2)
            g.scalar_tensor_tensor(c(6), s(10), -K, pb("khh"), op.mult, op.add)
            g.scalar_tensor_tensor(c(18), s(10), K, pb("khh"), op.mult, op.add)
            # b1p = -sx*stp: s8 = sx*stp -> P row = +K*s8, Q row = -K*s8
            g.tensor_tensor(s(8), s(4), pb("st"), op.mult)
            g.tensor_scalar(c(7), s(8), K, None, op.mult)
            g.tensor_scalar(c(19), s(8), -K, None, op.mult)
            # b2p = sy*ctp
            g.tensor_tensor(s(8), s(5), pb("ct"), op.mult)
            g.tensor_scalar(c(8), s(8), -K, None, op.mult)
            g.tensor_scalar(c(20), s(8), K, None, op.mult)
            # target box: dxt/dyt
            g.tensor_scalar(s(12), s(0), tq(j, "cx"), None, op.subtract)
            g.tensor_scalar(s(13), s(2), tq(j, "cy"), None, op.subtract)
            # a0t = dxt*ctt + dyt*stt
            g.tensor_scalar(s(8), s(12), tq(j, "ct"), None, op.mult)
            g.tensor_scalar(s(14), s(13), tq(j, "st"), None, op.mult)
            g.tensor_tensor(s(14), s(8), s(14), op.add)
            # b0t = dyt*ctt - dxt*stt
            g.tensor_scalar(s(8), s(13), tq(j, "ct"), None, op.mult)
            g.tensor_scalar(s(15), s(12), tq(j, "st"), None, op.mult)
            g.tensor_tensor(s(15), s(8), s(15), op.subtract)
            # field A2 const rows (rows 3-5/15-17)
            g.tensor_scalar(c(3), s(14), -K, tq(j, "khw"), op.mult, op.add)
            g.tensor_scalar(c(15), s(14), K, tq(j, "khw"), op.mult, op.add)
            # a1t = sx*ctt ; a2t = sy*stt
            g.tensor_scalar(s(8), s(4), tq(j, "ct"), None, op.mult)
            g.tensor_scalar(c(4), s(8), -K, None, op.mult)
            g.tensor_scalar(c(16), s(8), K, None, op.mult)
            g.tensor_scalar(s(8), s(5), tq(j, "st"), None, op.mult)
            g.tensor_scalar(c(5), s(8), -K, None, op.mult)
            g.tensor_scalar(c(17), s(8), K, None, op.mult)
            # field B2 const rows
            g.tensor_scalar(c(9), s(15), -K, tq(j, "khh"), op.mult, op.add)
            g.tensor_scalar(c(21), s(15), K, tq(j, "khh"), op.mult, op.add)
            # b1t = -sx*stt ; b2t = sy*ctt
            g.tensor_scalar(s(8), s(4), tq(j, "st"), None, op.mult)
            g.tensor_scalar(c(10), s(8), K, None, op.mult)
            g.tensor_scalar(c(22), s(8), -K, None, op.mult)
            g.tensor_scalar(s(8), s(5), tq(j, "ct"), None, op.mult)
            g.tensor_scalar(c(11), s(8), -K, None, op.mult)
            g.tensor_scalar(c(23), s(8), K, None, op.mult)

            Ssum = accp.tile([128, NLOC], dt.float32, tag="Ssum")
            Isum = accp.tile([128, NLOC], dt.float32, tag="Isum")

            # ---- main loop over the 64 predicted boxes of this core ----
            for n in range(NLOC):
                coeffT = psum.tile([24, 128], dt.float32, tag="coeffT")
                nc.tensor.transpose(coeffT[:], C[:, :, n], ident[:])
                lhsT = work.tile([24, 128], dt.float32, tag="lhsT")
                nc.vector.tensor_copy(lhsT[:], coeffT[:])

                fieldP = psum.tile([128, 4 * NPIX], dt.float32, tag="fields", bufs=3)
                fieldQ = psum.tile([128, 4 * NPIX], dt.float32, tag="fields", bufs=3)
                for q in range(2):
                    nc.tensor.matmul(
                        fieldP[:, q * 512:(q + 1) * 512],
                        lhsT[:], BAS[:, q * 512:(q + 1) * 512],
                        start=True, stop=True)
                for q in range(2):
                    nc.tensor.matmul(
                        fieldQ[:, q * 512:(q + 1) * 512],
                        lhsT[:], BAS[:, 1024 + q * 512:1024 + (q + 1) * 512],
                        start=True, stop=True)
                sigP = work.tile([128, 4 * NPIX], dt.bfloat16, tag="sigP")
                nc.scalar.activation(sigP[:], fieldP[:], AF.Sigmoid)
                sigQ = work.tile([128, 4 * NPIX], dt.bfloat16, tag="sigQ")
                nc.scalar.activation(sigQ[:], fieldQ[:], AF.Sigmoid)

                vmin = work.tile([128, 4, NPIX], dt.bfloat16, tag="vmin")
                nc.vector.tensor_tensor(
                    vmin[:].rearrange("p f q -> p (f q)"),
                    sigP[:], sigQ[:], op.min)

                vflat = vmin[:].rearrange("p f q -> p (f q)")
                Fp = work.tile([128, 2 * NPIX], dt.bfloat16, tag="Fp")
                nc.vector.tensor_mul(Fp[:], vflat[:, 0:2 * NPIX], vflat[:, 2 * NPIX:4 * NPIX])
                nc.vector.tensor_reduce(
                    Ssum[:, n:n + 1], Fp[:], mybir.AxisListType.X, op.add)
                F12 = work.tile([128, NPIX], dt.bfloat16, tag="F12")
                nc.vector.tensor_mul(F12[:], Fp[:, 0:NPIX], Fp[:, NPIX:2 * NPIX])
                nc.vector.tensor_reduce(
                    Isum[:, n:n + 1], F12[:], mybir.AxisListType.X, op.add)

            # ---- epilogue: piou = inter / (stot - inter + eps) ----
            union = scratch.tile([128, NLOC], dt.float32, tag="union")
            nc.vector.scalar_tensor_tensor(
                union[:], Isum[:], -1.0, Ssum[:], op.mult, op.add)
            nc.vector.tensor_scalar(union[:], union[:], float(EPS), None, op.add)
            rec = scratch.tile([128, NLOC], dt.float32, tag="rec")
            nc.vector.reciprocal(rec[:], union[:])
            piou = accp.tile([128, NLOC], dt.float16, tag="piou")
            nc.vector.tensor_tensor(piou[:], Isum[:], rec[:], op.mult)
            nc.sync.dma_start(out=OUT_d[j * 128:(j + 1) * 128, :], in_=piou[:])

    nc.finalize()
    return nc


def _get_runner():
    """Build (once) a reusable jitted PJRT dispatch for the Bass module.

    Re-jitting per call (what run_bass_kernel_spmd does under axon) costs
    ~400ms of retrace + XLA recompile every call; here the jitted callable,
    the device-resident BASIS constant, and the structural zero-output
    buffers all persist across calls.
    """
    if "runner" in _cache:
        return _cache["runner"]

    import jax
    from jax.sharding import Mesh, NamedSharding, PartitionSpec
    from jax.experimental.shard_map import shard_map  # check_rep kwarg
    from concourse import mybir
    from concourse.bass2jax import (
        _bass_exec_p,
        install_neuronx_cc_hook,
        partition_id_tensor,
    )

    nc = _build_nc()
    install_neuronx_cc_hook()
    partition_name = nc.partition_id_tensor.name if nc.partition_id_tensor else None

    in_names, out_names, out_avals = [], [], []
    for alloc in nc.m.functions[0].allocations:
        if not isinstance(alloc, mybir.MemoryLocationSet):
            continue
        name = alloc.memorylocations[0].name
        if alloc.kind == "ExternalInput":
            if name != partition_name:
                in_names.append(name)
        elif alloc.kind == "ExternalOutput":
            out_names.append(name)
            out_avals.append(
                jax.core.ShapedArray(tuple(alloc.tensor_shape),
                                     mybir.dt.np(alloc.dtype)))
    assert in_names == ["X", "BASIS"] and out_names == ["OUT"], (in_names, out_names)

    all_in_names = list(in_names) + list(out_names)
    if partition_name is not None:
        all_in_names.append(partition_name)

    def _body(*args):
        operands = list(args)
        if partition_name is not None:
            operands.append(partition_id_tensor())
        return tuple(_bass_exec_p.bind(
            *operands,
            out_avals=tuple(out_avals),
            in_names=tuple(all_in_names),
            out_names=tuple(out_names),
            lowering_input_output_aliases=(),
            sim_require_finite=True,
            sim_require_nnan=True,
            nc=nc,
        ))

    mesh = Mesh(np.asarray(jax.devices()[:NC]), ("core",))
    spec = PartitionSpec("core")
    n_ops = len(in_names) + len(out_names)
    fn = jax.jit(
        shard_map(_body, mesh=mesh, in_specs=(spec,) * n_ops,
                  out_specs=(spec,) * len(out_names), check_rep=False),
        keep_unused=True,
    )

    sh = NamedSharding(mesh, spec)
    # Device-resident across calls: the constant basis (replicated per core)
    # and the zero buffer backing the OUT operand slot.  The NEFF never
    # reads or writes this operand (the renamed NEFF binds OUT only as
    # output0, and the kernel writes every OUT element), so one buffer can
    # be reused every call without donation.
    bas_dev = jax.device_put(
        np.broadcast_to(_basis(), (NC, 24, 8 * NPIX)).reshape(NC * 24, 8 * NPIX), sh)
    zout_dev = jax.device_put(np.zeros((NC * M, NLOC), np.float16), sh)

    def run(x_packed):
        try:
            out, = fn(x_packed, bas_dev, zout_dev)
            return np.asarray(out)  # [NC*M, NLOC] fp16
        except Exception:
            # One retry for transient tunnel/runtime hiccups; a persistent
            # device wedge will re-raise.
            out, = fn(x_packed, bas_dev, zout_dev)
            return np.asarray(out)

    # Warm: pay NEFF compile + jit trace + device layout here, not in the
    # first timed call.
    run(np.zeros((NC * XROWS, 40), np.float32))

    _cache["runner"] = run
    return run


def kernel(loc_p, loc_t, grid):
    assert int(grid) == G
    loc_p = np.asarray(loc_p, dtype=np.float32)
    loc_t = np.asarray(loc_t, dtype=np.float32)

    run = _get_runner()
    res = run(_pack_inputs(loc_p, loc_t))  # [NC*M, NLOC] fp16
    # res[c*M + m, n] = piou(box c*NLOC+n, box m)  ->  out[n_global, m]
    out = res.reshape(NC, M, NLOC).transpose(0, 2, 1).reshape(N, M)
    return np.ascontiguousarray(out, dtype=np.float32)


# revision 29
# speedup vs baseline: 9.4890x; 1.0270x over previous
"""PIoU (pixel-wise IoU) pairwise matrix kernel for Trainium2, 8 NeuronCores.

Math: for each pair (predicted box n, target box m) the reference samples a
16x16 grid of the joint AABB and evaluates a soft membership
F = sigmoid(k(w/2-|A|)) * sigmoid(k(h/2-|B|)) per box, where (A, B) are the
pixel offsets rotated into the box frame.  Both A and B are *affine* in the
grid coordinates (ug, uh), so the sigmoid arguments k(s/2 -+ A) for all
256 pixels x 4 fields x {P,Q} are produced by ONE K=24 matmul per 128 pairs
against a constant basis.  sigmoid(min(P,Q)) == min(sigmoid(P), sigmoid(Q))
lets ACT read the matmul PSUM directly with no bias work.

Sharding: N (predicted) axis split 8 ways; each core computes a [512m, 64n]
slab (output transposed on host).

Dispatch: the per-call cost is dominated by the axon tunnel round trip
(~80ms) plus payload bytes, so the runner (a) jits the PJRT shard_map
function ONCE and reuses it every call, (b) keeps the constant basis and
the structural zero-output buffers resident on device, and (c) ships only
one packed [144,40] f32 tensor per core per call (~184KB total) with the
per-box broadcast done on device via a K=1 matmul against an all-ones
basis row.  Output returns as fp16 (512KB) and is upcast on host.
"""

import numpy as np

N = 512
M = 512
G = 16
NPIX = G * G
K_SLOPE = np.float32(10.0)
EPS = np.float32(1e-6)
NC = 8
NLOC = N // NC  # 64 predicted boxes per core
NCHUNK = 4  # m-chunks of 128
XROWS = 144  # packed input: rows 0..127 = TQ [128,40], rows 128..143 = PB [16,40]

_cache = {}

_Q_ORDER = ("x0", "x1", "y0", "y1", "cx", "cy", "ct", "st", "khw", "khh")


def _derived(b):
    # b: [K,5] float32 -> per-box derived quantities (all float32)
    cx, cy, w, h, t = (b[:, i].astype(np.float32) for i in range(5))
    c, s = np.cos(t).astype(np.float32), np.sin(t).astype(np.float32)
    hw = np.float32(0.5) * (w * np.abs(c) + h * np.abs(s))
    hh = np.float32(0.5) * (w * np.abs(s) + h * np.abs(c))
    return dict(
        cx=cx, cy=cy, ct=c, st=s,
        khw=(K_SLOPE * np.float32(0.5)) * w, khh=(K_SLOPE * np.float32(0.5)) * h,
        x0=cx - hw, x1=cx + hw, y0=cy - hh, y1=cy + hh,
    )


def _basis():
    """Constant [24, 2*NPIX*4] sampling basis (shipped to device once)."""
    if "basis" in _cache:
        return _cache["basis"]
    u = ((np.arange(G, dtype=np.float32) + np.float32(0.5)) / np.float32(G))
    Ug = np.tile(u, G)      # pixel p = h*G+g -> u[g]
    Uh = np.repeat(u, G)    # -> u[h]

    # basis [24, 2*NPIX*4]: P-block cols 0..1023 (fields A1,B1,A2,B2 x 256),
    # Q-block cols 1024..2047.  Field f uses rows 3f..3f+2 (P) / 12+3f.. (Q).
    # Row 0 is all-ones over cols 0..255; its first 128 cols double as the
    # lhsT for the on-device partition broadcast of the per-box rows.
    basis = np.zeros((24, 8 * NPIX), dtype=np.float32)
    for f in range(4):
        for blk, r0 in ((0, 0), (1, 12)):
            c0 = blk * 4 * NPIX + f * NPIX
            basis[r0 + 3 * f + 0, c0:c0 + NPIX] = 1.0
            basis[r0 + 3 * f + 1, c0:c0 + NPIX] = Ug
            basis[r0 + 3 * f + 2, c0:c0 + NPIX] = Uh
    _cache["basis"] = basis
    return basis


def _pack_inputs(loc_p, loc_t):
    """Per-call packed input [NC*XROWS, 40]: TQ (replicated) + per-core PB."""
    T = _derived(loc_t)
    # TQ [128, 4 chunks, 10]: per-target quantities, m = j*128 + partition
    TQ = np.empty((128, NCHUNK, len(_Q_ORDER)), dtype=np.float32)
    for qi, q in enumerate(_Q_ORDER):
        TQ[:, :, qi] = T[q].reshape(NCHUNK, 128).T
    TQflat = TQ.reshape(128, NCHUNK * len(_Q_ORDER))

    P = _derived(loc_p)
    X = np.empty((NC, XROWS, 40), dtype=np.float32)
    for c in range(NC):
        sl = slice(c * NLOC, (c + 1) * NLOC)
        pb = np.stack([P[q][sl] for q in _Q_ORDER], axis=0)  # [10, 64]
        X[c, 0:128, :] = TQflat
        X[c, 128:XROWS, :] = pb.reshape(16, 40)
    return X.reshape(NC * XROWS, 40)


def _build_nc():
    from contextlib import ExitStack

    import concourse.bacc as bacc
    import concourse.tile as tile
    from concourse import mybir
    from concourse.masks import make_identity

    dt = mybir.dt
    op = mybir.AluOpType
    AF = mybir.ActivationFunctionType
    K = float(K_SLOPE)

    # Bacc (not raw Bass): its finalize() runs generate_event_semaphores,
    # which legalizes Tile's multi-wait sync_info down to <=1 wait per
    # hardware instruction.
    nc = bacc.Bacc(None, target_bir_lowering=False)
    X_d = nc.declare_dram_parameter("X", [XROWS, 40], dt.float32, isOutput=False)
    BAS_d = nc.declare_dram_parameter("BASIS", [24, 8 * NPIX], dt.float32, isOutput=False)
    OUT_d = nc.declare_dram_parameter("OUT", [M, NLOC], dt.float16, isOutput=True)

    with tile.TileContext(nc) as tc, ExitStack() as ctx:
        consts = ctx.enter_context(tc.tile_pool(name="consts", bufs=1))
        coeffp = ctx.enter_context(tc.tile_pool(name="coeffp", bufs=2))
        scratch = ctx.enter_context(tc.tile_pool(name="scratch", bufs=2))
        work = ctx.enter_context(tc.tile_pool(name="work", bufs=2))
        accp = ctx.enter_context(tc.tile_pool(name="accp", bufs=2))
        psum = ctx.enter_context(tc.tile_pool(name="psum", bufs=2, space="PSUM"))

        ident = consts.tile([128, 128], dt.float32)
        make_identity(nc, ident[:])
        TQ = consts.tile([128, NCHUNK, 10], dt.float32)
        nc.sync.dma_start(out=TQ[:].rearrange("p a b -> p (a b)"), in_=X_d[0:128, :])
        pbrow = consts.tile([1, 10 * NLOC], dt.float32)
        nc.sync.dma_start(
            out=pbrow[:],
            in_=X_d[128:XROWS, :].rearrange("a b -> (a b)").unsqueeze(0))
        BAS = consts.tile([24, 8 * NPIX], dt.float32)
        nc.sync.dma_start(out=BAS[:], in_=BAS_d[:])

        # Broadcast the 640 per-box values to all 128 partitions with a K=1
        # matmul: BAS row 0 is all-ones on cols 0..255, so BAS[0:1, 0:128]
        # is a ready-made ones lhsT.  PSUM reuses one "fields" ring slot.
        PB = consts.tile([128, 10, NLOC], dt.float32)
        PBflat = PB[:].rearrange("p a b -> p (a b)")
        bc = psum.tile([128, 4 * NPIX], dt.float32, tag="fields", bufs=3)
        nc.tensor.matmul(bc[:, 0:512], BAS[0:1, 0:128], pbrow[:, 0:512],
                         start=True, stop=True)
        nc.tensor.matmul(bc[:, 512:640], BAS[0:1, 0:128], pbrow[:, 512:640],
                         start=True, stop=True)
        nc.vector.tensor_copy(PBflat[:, 0:640], bc[:, 0:640])

        def pb(q):
            return PB[:, _Q_ORDER.index(q), :]

        def tq(j, q):
            i = _Q_ORDER.index(q)
            return TQ[:, j, i:i + 1]

        for j in range(NCHUNK):
            # ---- coefficient slab C [128 m, 24 rows, 64 n] on DVE ----
            C = coeffp.tile([128, 24, NLOC], dt.float32, tag="C")
            S = scratch.tile([128, 16, NLOC], dt.float32, tag="S")
            g = nc.vector

            def s(i):
                return S[:, i, :]

            g.tensor_scalar(s(0), pb("x0"), tq(j, "x0"), None, op.min)   # xmin
            g.tensor_scalar(s(1), pb("x1"), tq(j, "x1"), None, op.max)   # xmax
            g.tensor_scalar(s(2), pb("y0"), tq(j, "y0"), None, op.min)   # ymin
            g.tensor_scalar(s(3), pb("y1"), tq(j, "y1"), None, op.max)   # ymax
            g.tensor_tensor(s(4), s(1), s(0), op.subtract)               # sx
            g.tensor_tensor(s(5), s(3), s(2), op.subtract)               # sy
            g.tensor_tensor(s(6), s(0), pb("cx"), op.subtract)           # dxp
            g.tensor_tensor(s(7), s(2), pb("cy"), op.subtract)           # dyp
            # a0p = dxp*ctp + dyp*stp ; b0p = dyp*ctp - dxp*stp
            g.tensor_tensor(s(8), s(6), pb("ct"), op.mult)
            g.tensor_tensor(s(9), s(7), pb("st"), op.mult)
            g.tensor_tensor(s(9), s(8), s(9), op.add)                    # a0p
            g.tensor_tensor(s(8), s(7), pb("ct"), op.mult)
            g.tensor_tensor(s(10), s(6), pb("st"), op.mult)
            g.tensor_tensor(s(10), s(8), s(10), op.subtract)             # b0p

            def c(r):
                return C[:, r, :]

            # field A1 (const rows): P = khw_p - K*a0p ; Q = khw_p + K*a0p
            g.scalar_tensor_tensor(c(0), s(9), -K, pb("khw"), op.mult, op.add)
            g.scalar_tensor_tensor(c(12), s(9), K, pb("khw"), op.mult, op.add)
            # a1p = sx*ctp -> rows 1/13 ; a2p = sy*stp -> rows 2/14
            g.tensor_tensor(s(8), s(4), pb("ct"), op.mult)
            g.tensor_scalar(c(1), s(8), -K, None, op.mult)
            g.tensor_scalar(c(13), s(8), K, None, op.mult)
            g.tensor_tensor(s(8), s(5), pb("st"), op.mult)
            g.tensor_scalar(c(2), s(8), -K, None, op.mult)
            g.tensor_scalar(c(14), s(8), K, None, op.mult)
            # field B1 (rows 6-8/18-20; field order is A1,A2,B1,B2)
            g.scalar_tensor_tensor(c(6), s(10), -K, pb("khh"), op.mult, op.add)
            g.scalar_tensor_tensor(c(18), s(10), K, pb("khh"), op.mult, op.add)
            # b1p = -sx*stp: s8 = sx*stp -> P row = +K*s8, Q row = -K*s8
            g.tensor_tensor(s(8), s(4), pb("st"), op.mult)
            g.tensor_scalar(c(7), s(8), K, None, op.mult)
            g.tensor_scalar(c(19), s(8), -K, None, op.mult)
            # b2p = sy*ctp
            g.tensor_tensor(s(8), s(5), pb("ct"), op.mult)
            g.tensor_scalar(c(8), s(8), -K, None, op.mult)
            g.tensor_scalar(c(20), s(8), K, None, op.mult)
            # target box: dxt/dyt
            g.tensor_scalar(s(12), s(0), tq(j, "cx"), None, op.subtract)
            g.tensor_scalar(s(13), s(2), tq(j, "cy"), None, op.subtract)
            # a0t = dxt*ctt + dyt*stt
            g.tensor_scalar(s(8), s(12), tq(j, "ct"), None, op.mult)
            g.tensor_scalar(s(14), s(13), tq(j, "st"), None, op.mult)
            g.tensor_tensor(s(14), s(8), s(14), op.add)
            # b0t = dyt*ctt - dxt*stt
            g.tensor_scalar(s(8), s(13), tq(j, "ct"), None, op.mult)
            g.tensor_scalar(s(15), s(12), tq(j, "st"), None, op.mult)
            g.tensor_tensor(s(15), s(8), s(15), op.subtract)
            # field A2 const rows (rows 3-5/15-17)
            g.tensor_scalar(c(3), s(14), -K, tq(j, "khw"), op.mult, op.add)
            g.tensor_scalar(c(15), s(14), K, tq(j, "khw"), op.mult, op.add)
            # a1t = sx*ctt ; a2t = sy*stt
            g.tensor_scalar(s(8), s(4), tq(j, "ct"), None, op.mult)
            g.tensor_scalar(c(4), s(8), -K, None, op.mult)
            g.tensor_scalar(c(16), s(8), K, None, op.mult)
            g.tensor_scalar(s(8), s(5), tq(j, "st"), None, op.mult)
            g.tensor_scalar(c(5), s(8), -K, None, op.mult)
            g.tensor_scalar(c(17), s(8), K, None, op.mult)
            # field B2 const rows
            g.tensor_scalar(c(9), s(15), -K, tq(j, "khh"), op.mult, op.add)
            g.tensor_scalar(c(21), s(15), K, tq(j, "khh"), op.mult, op.add)
            # b1t = -sx*stt ; b2t = sy*ctt
            g.tensor_scalar(s(8), s(4), tq(j, "st"), None, op.mult)
            g.tensor_scalar(c(10), s(8), K, None, op.mult)
            g.tensor_scalar(c(22), s(8), -K, None, op.mult)
            g.tensor_scalar(s(8), s(5), tq(j, "ct"), None, op.mult)
            g.tensor_scalar(c(11), s(8), -K, None, op.mult)
            g.tensor_scalar(c(23), s(8), K, None, op.mult)

            Ssum = accp.tile([128, NLOC], dt.float32, tag="Ssum")
            Isum = accp.tile([128, NLOC], dt.float32, tag="Isum")

            # ---- main loop over the 64 predicted boxes of this core ----
            for n in range(NLOC):
                coeffT = psum.tile([24, 128], dt.float32, tag="coeffT")
                nc.tensor.transpose(coeffT[:], C[:, :, n], ident[:])
                lhsT = work.tile([24, 128], dt.float32, tag="lhsT")
                nc.vector.tensor_copy(lhsT[:], coeffT[:])

                fieldP = psum.tile([128, 4 * NPIX], dt.float32, tag="fields", bufs=3)
                fieldQ = psum.tile([128, 4 * NPIX], dt.float32, tag="fields", bufs=3)
                for q in range(2):
                    nc.tensor.matmul(
                        fieldP[:, q * 512:(q + 1) * 512],
                        lhsT[:], BAS[:, q * 512:(q + 1) * 512],
                        start=True, stop=True)
                for q in range(2):
                    nc.tensor.matmul(
                        fieldQ[:, q * 512:(q + 1) * 512],
                        lhsT[:], BAS[:, 1024 + q * 512:1024 + (q + 1) * 512],
                        start=True, stop=True)
                sigP = work.tile([128, 4 * NPIX], dt.bfloat16, tag="sigP")
                nc.scalar.activation(sigP[:], fieldP[:], AF.Sigmoid)
                sigQ = work.tile([128, 4 * NPIX], dt.bfloat16, tag="sigQ")
                nc.scalar.activation(sigQ[:], fieldQ[:], AF.Sigmoid)

                vmin = work.tile([128, 4, NPIX], dt.bfloat16, tag="vmin")
                nc.vector.tensor_tensor(
                    vmin[:].rearrange("p f q -> p (f q)"),
                    sigP[:], sigQ[:], op.min)

                vflat = vmin[:].rearrange("p f q -> p (f q)")
                Fp = work.tile([128, 2 * NPIX], dt.bfloat16, tag="Fp")
                nc.vector.tensor_mul(Fp[:], vflat[:, 0:2 * NPIX], vflat[:, 2 * NPIX:4 * NPIX])
                nc.vector.tensor_reduce(
                    Ssum[:, n:n + 1], Fp[:], mybir.AxisListType.X, op.add)
                F12 = work.tile([128, NPIX], dt.bfloat16, tag="F12")
                nc.vector.tensor_mul(F12[:], Fp[:, 0:NPIX], Fp[:, NPIX:2 * NPIX])
                nc.vector.tensor_reduce(
                    Isum[:, n:n + 1], F12[:], mybir.AxisListType.X, op.add)

            # ---- epilogue: piou = inter / (stot - inter + eps) ----
            union = scratch.tile([128, NLOC], dt.float32, tag="union")
            nc.vector.scalar_tensor_tensor(
                union[:], Isum[:], -1.0, Ssum[:], op.mult, op.add)
            nc.vector.tensor_scalar(union[:], union[:], float(EPS), None, op.add)
            rec = scratch.tile([128, NLOC], dt.float32, tag="rec")
            nc.vector.reciprocal(rec[:], union[:])
            piou = accp.tile([128, NLOC], dt.float16, tag="piou")
            nc.vector.tensor_tensor(piou[:], Isum[:], rec[:], op.mult)
            nc.sync.dma_start(out=OUT_d[j * 128:(j + 1) * 128, :], in_=piou[:])

    nc.finalize()
    return nc


def _get_runner():
    """Build (once) a reusable jitted PJRT dispatch for the Bass module.

    Re-jitting per call (what run_bass_kernel_spmd does under axon) costs
    ~400ms of retrace + XLA recompile every call; here the jitted callable,
    the device-resident BASIS constant, and the structural zero-output
    buffers all persist across calls.
    """
    if "runner" in _cache:
        return _cache["runner"]

    import jax
    from jax.sharding import Mesh, NamedSharding, PartitionSpec
    from jax.experimental.shard_map import shard_map  # check_rep kwarg
    from concourse import mybir
    from concourse.bass2jax import (
        _bass_exec_p,
        install_neuronx_cc_hook,
        partition_id_tensor,
    )

    nc = _build_nc()
    install_neuronx_cc_hook()
    partition_name = nc.partition_id_tensor.name if nc.partition_id_tensor else None

    in_names, out_names, out_avals = [], [], []
    for alloc in nc.m.functions[0].allocations:
        if not isinstance(alloc, mybir.MemoryLocationSet):
            continue
        name = alloc.memorylocations[0].name
        if alloc.kind == "ExternalInput":
            if name != partition_name:
                in_names.append(name)
        elif alloc.kind == "ExternalOutput":
            out_names.append(name)
            out_avals.append(
                jax.core.ShapedArray(tuple(alloc.tensor_shape),
                                     mybir.dt.np(alloc.dtype)))
    assert in_names == ["X", "BASIS"] and out_names == ["OUT"], (in_names, out_names)

    all_in_names = list(in_names) + list(out_names)
    if partition_name is not None:
        all_in_names.append(partition_name)

    def _body(*args):
        operands = list(args)
        if partition_name is not None:
            operands.append(partition_id_tensor())
        return tuple(_bass_exec_p.bind(
            *operands,
            out_avals=tuple(out_avals),
            in_names=tuple(all_in_names),
            out_names=tuple(out_names),
            lowering_input_output_aliases=(),
            sim_require_finite=True,
            sim_require_nnan=True,
            nc=nc,
        ))

    mesh = Mesh(np.asarray(jax.devices()[:NC]), ("core",))
    spec = PartitionSpec("core")
    n_ops = len(in_names) + len(out_names)
    fn = jax.jit(
        shard_map(_body, mesh=mesh, in_specs=(spec,) * n_ops,
                  out_specs=(spec,) * len(out_names), check_rep=False),
        keep_unused=True,
    )

    sh = NamedSharding(mesh, spec)
    # Device-resident across calls: the constant basis (replicated per core)
    # and the zero buffer backing the OUT operand slot.  The NEFF never
    # reads or writes this operand (the renamed NEFF binds OUT only as
    # output0, and the kernel writes every OUT element), so one buffer can
    # be reused every call without donation.
    bas_dev = jax.device_put(
        np.broadcast_to(_basis(), (NC, 24, 8 * NPIX)).reshape(NC * 24, 8 * NPIX), sh)
    zout_dev = jax.device_put(np.zeros((NC * M, NLOC), np.float16), sh)

    def run(x_packed):
        try:
            out, = fn(x_packed, bas_dev, zout_dev)
            return np.asarray(out)  # [NC*M, NLOC] fp16
        except Exception:
            # One retry for transient tunnel/runtime hiccups; a persistent
            # device wedge will re-raise.
            out, = fn(x_packed, bas_dev, zout_dev)
            return np.asarray(out)

    # Warm: pay NEFF compile + jit trace + device layout here, not in the
    # first timed call.
    run(np.zeros((NC * XROWS, 40), np.float32))

    _cache["runner"] = run
    return run


def kernel(loc_p, loc_t, grid):
    assert int(grid) == G
    loc_p = np.asarray(loc_p, dtype=np.float32)
    loc_t = np.asarray(loc_t, dtype=np.float32)

    run = _get_runner()
    res = run(_pack_inputs(loc_p, loc_t))  # [NC*M, NLOC] fp16
    # res[c*M + m, n] = piou(box c*NLOC+n, box m)  ->  out[n_global, m]
    out = res.reshape(NC, M, NLOC).transpose(0, 2, 1).reshape(N, M)
    return np.ascontiguousarray(out, dtype=np.float32)
